# revision 37
# baseline (speedup 1.0000x reference)
"""AffinityPropagate Trainium2 kernel.

Problem: 24 iterations of a per-pixel-weighted 3x3 stencil (zero-padded)
on a [B=8, C=1, H=256, W=1216] image, weights = abs-normalized affinity
[B, 9, H, W].  Data-parallel over batch: one image per NeuronCore.

Per-core algorithm
------------------
Row i of the image maps to (partition p = i//2, slot c = i%2), so the
256 rows live on 128 partitions x 2 free-dim blocks.  With that mapping
a +-1 row shift never crosses a partition-chunk seam: it is a partition
shift by one (handled exactly by a 128x128 shifted-identity matmul,
zero-padding falls out of the missing matrix row) and/or a slot swap
(pure free-dim addressing).

Weights are normalized once, then pre-shifted so the per-iteration inner
loop is only:
  DVE : z[n]  = wsh[n] * f          (9 taps in one tensor_tensor via a
                                     stride-0 broadcast of f; fp16 -> 2x mode.
                                     Pool offload was tried and reverted: on
                                     real HW concurrent Pool tensor ops halve
                                     DVE's 2x throughput -- shared SBUF
                                     bandwidth -- so any offload is net lose.)
  PE  : out[c] += S_{n,c} @ z[n][shifted columns]   (PSUM fp32 accumulate)
  ACT : f' = cast(out)              (PSUM -> SBUF fp16 copy)
All column (W) shifts are plain free-dim address offsets into z's
guard-padded blocks; all row (H) shifts are the stationary matrices.

Preamble: |affinity| is taken host-side (same staging class as the fp16
cast), and ships band-major: one DMA per PSUM-bank column band, each
band self-contained with a 1-column overlap on both sides (edge bands
duplicate the image border column; those pad columns are never read
back).  Each band's normalize (PE identity-matmul map-sum -> DVE
reciprocal -> normalize/pre-shift on DVE -> shift matmuls) and
iteration 1's TT start as soon as that band's DMA lands, so the loop is
running while later bands are still in flight.  Band 0's DMA ships in
three 3-map chunks chased by the normalizer matmuls; a PE warm-up
(idle matmuls on the early-landed shift matrices) keeps the clock
ramped; and iteration 1's tile-0 matmul phases interleave into the
band pipeline (reusing band 0's freed PSUM banks) so iteration 2 never
stalls on its evacuation.  The final iteration halves its last tile to
shorten the trailing matmul/evac/store chain.
"""

import os
import sys

import numpy as np

for _p in ("/opt/trn_rl_repo", "/opt/pypackages"):
    if os.path.isdir(_p) and _p not in sys.path:
        sys.path.insert(0, _p)

B, K2, H, W = 8, 9, 256, 1216
P = 128          # partitions; row i -> (p=i//2, c=i%2)
NS = 2           # row slots per partition
GU = 2           # guard columns on each side of a z block (4B aligned)
WB = W + 2 * GU  # z/wsh block width
TILES = [(0, 406), (406, 406), (812, 404)]  # PSUM bank tiling (even sizes)
# TT tiling matches the PSUM tiling: iteration k+1's first TT needs only
# f_next tile 0, which ACT evacuates while tiles 1-2 of iteration k are
# still in flight.  (A coarser TT split serializes on that handoff.)
TT_TILES = TILES
# The final iteration halves its last tile so the trailing
# matmul+evacuate+store chain after the very last TT is half as long
# (the first half's output DMA overlaps the second half's matmuls).
LAST_TILES = [(0, 406), (406, 406), (812, 270), (1082, 134)]

# Per-bank matmul plan: out slot c' accumulates, for each tap n,
# z[n][src slot] routed through stationary matrix:
#   s = di-1 = -1: c'=0 <- (S_dn, slot 1),  c'=1 <- (I, slot 0)
#   s = 0        : c' <- (I, slot c')
#   s = +1       : c'=0 <- (I, slot 1),  c'=1 <- (S_up, slot 0)
# Stationary index: 0 = S_dn (k == m-1), 1 = I, 2 = S_up (k == m+1).
# Ordered so consecutive matmuls mostly share the stationary operand.
PLAN = {
    0: [(0, 1, 0), (1, 1, 0), (2, 1, 0),
        (3, 0, 1), (4, 0, 1), (5, 0, 1),
        (6, 1, 1), (7, 1, 1), (8, 1, 1)],
    1: [(0, 0, 1), (1, 0, 1), (2, 0, 1),
        (3, 1, 1), (4, 1, 1), (5, 1, 1),
        (6, 0, 2), (7, 0, 2), (8, 0, 2)],
}

# Taps with a row shift (di != 1).
SHIFT_TAPS = (0, 1, 2, 6, 7, 8)

_CACHE = {}


def _shift_mats() -> np.ndarray:
    """[128, 3, 128] fp16 (SBUF layout [k, which, m]): S_dn (k==m-1), I,
    S_up (k==m+1) as lhsT[k, m]."""
    s = np.zeros((3, P, P), dtype=np.float16)
    k = np.arange(P - 1)
    s[0][k, k + 1] = 1.0      # out[m] = mov[m-1]
    s[1][np.arange(P), np.arange(P)] = 1.0
    s[2][k + 1, k] = 1.0      # out[m] = mov[m+1]
    return np.ascontiguousarray(s.transpose(1, 0, 2))


def build_program(times: int):
    import concourse.bacc as bacc
    import concourse.tile as tile
    from concourse import mybir
    from contextlib import ExitStack

    dt = mybir.dt
    nc = bacc.Bacc(trn_type="TRN2", target_bir_lowering=False, debug=False,
                   num_devices=B)

    # |a| ships already transposed to the SBUF layout [p, n, c, j], in
    # three column bands (1-col overlap each side): each band is one
    # contiguous DRAM run per partition -> max DMA efficiency, and each
    # band's normalize pipeline is independent of the others.
    aff_b = []
    for bi, (t0, L) in enumerate(TILES):
        aff_b.append(nc.dram_tensor(f"aff{bi}", [P, K2, NS, L + 2],
                                    dt.float16, kind="ExternalInput"))
    feat = nc.dram_tensor("feat", [H, W], dt.float16, kind="ExternalInput")
    # smat ships pre-transposed to the SBUF layout [p, 3, k] (contiguous
    # per-partition lines; a transposed DMA of the [3,P,P] layout has
    # 256B elements and crawls).
    smat = nc.dram_tensor("smat", [P, 3, P], dt.float16, kind="ExternalInput")
    out = nc.dram_tensor("out", [H, W], dt.float16, kind="ExternalOutput")

    feat_r = feat.ap().rearrange("(p c) j -> p c j", c=NS)
    out_r = out.ap().rearrange("(p c) j -> p c j", c=NS)

    with tile.TileContext(nc) as tc, ExitStack() as ctx:
        persist = ctx.enter_context(tc.tile_pool(name="persist", bufs=1))
        # bufs=3: with 2, iteration k+1's f-tile allocation WARs on
        # iteration k-1's last TT read and stalls DVE ~1.5us at the
        # preamble/loop seam.
        fpool = ctx.enter_context(tc.tile_pool(name="fpool", bufs=3))

        smat_t = persist.tile([P, 3, P], dt.float16)
        wsh = persist.tile([P, K2, NS, WB], dt.float16)
        # z buffer 0 lives outside the preamble pools so iteration 1's TT
        # does not wait for the preamble-pool release.  z guard columns are
        # never initialized: the edge matmuls clamp their column ranges
        # instead (the clamped-away taps are exactly the zero-padding).
        z0 = persist.tile([P, K2, NS, WB], dt.float16, name="z0")

        f0 = fpool.tile([P, NS, W], dt.float16, tag="f")

        # ---- loop emission helpers (iteration 1's TTs are interleaved
        # with the preamble build below) ----
        dj01 = {c: [e for e in PLAN[c] if e[0] % 3 <= 1] for c in (0, 1)}
        dj2 = {c: [e for e in PLAN[c] if e[0] % 3 == 2] for c in (0, 1)}
        # Tile 0 starts its accumulation group with a dj=1 tap (full column
        # range) so the edge-clamped dj=0 taps never own the start flag;
        # order keeps stationary runs contiguous (no extra LDWEIGHTS).
        tap_order = {0: [4, 3, 6, 7, 0, 1], 1: [4, 3, 0, 1, 6, 7]}
        dj01_t0 = {c: sorted(dj01[c], key=lambda e: tap_order[c].index(e[0]))
                   for c in (0, 1)}

        def emit_tt(z, f_cur, t0, L):
            fb = f_cur[:, :, t0:t0 + L].unsqueeze(1).broadcast_to(
                [P, K2, NS, L])
            nc.vector.tensor_tensor(
                out=z[:, :, :, GU + t0:GU + t0 + L],
                in0=wsh[:, :, :, GU + t0:GU + t0 + L],
                in1=fb, op=mybir.AluOpType.mult)

        # Shared matmul emitter (used by the steady loop AND the
        # preamble-interleaved first iteration).
        def emit_mms(z, ps, c, t0, L, entries, start, stop):
            # At the image's W edges the out-of-range tap column is
            # dropped (zero padding) instead of reading a z guard cell,
            # so z guards never need initializing.
            for mi, (n, c_src, sidx) in enumerate(entries):
                dj = n % 3
                m0 = GU + t0 + dj - 1
                o0 = 1 if (t0 == 0 and dj == 0) else 0
                o1 = L - 1 if (t0 + L == W and dj == 2) else L
                nc.tensor.matmul(
                    out=ps[:, o0:o1], lhsT=smat_t[:, sidx, :],
                    rhs=z[:, n, c_src, m0 + o0:m0 + o1],
                    start=start and mi == 0,
                    stop=stop and mi == len(entries) - 1,
                    skip_group_check=True)

        with tc.tile_pool(name="pre", bufs=1) as prep, \
                tc.tile_pool(name="accps", bufs=6, space="PSUM") as accp, \
                tc.tile_pool(name="prepsum", bufs=2, space="PSUM") as prepsum:
            # Per-band preamble tiles (all three bands coexist; each is
            # consumed by its own band's build only).
            absa, rcp32, rcp, ws03, ws68, acc = [], [], [], [], [], {}
            for bi, (t0, L) in enumerate(TILES):
                lb = L + 2
                absa.append(prep.tile([P, K2, NS, lb], dt.float16,
                                      name=f"absa{bi}"))
                rcp32.append(prep.tile([P, NS, lb], dt.float32,
                                       name=f"rcp32_{bi}"))
                # 3 slots: [s0, s1, s0] — the duplicated s0 lets the
                # paired direct ops read (s1, s0) as a plain +1-stride dim.
                rcp.append(prep.tile([P, 3, lb], dt.float16,
                                     name=f"rcp{bi}"))
                ws03.append(prep.tile([P, 3, lb], dt.float16,
                                      name=f"ws03_{bi}"))
                ws68.append(prep.tile([P, 3, lb], dt.float16,
                                      name=f"ws68_{bi}"))
                for s in range(NS):
                    acc[(bi, s)] = accp.tile([P, lb], dt.float32, tag="acc",
                                             name=f"acc_{bi}_{s}")

            # wsh cells the build never writes (guards + the one edge
            # column of each shifted tap) must be zero: they feed the loop
            # TT and the shift matmuls would accumulate garbage otherwise.
            nc.vector.memset(wsh[:, :, :, 0:GU + 1], 0.0)
            nc.vector.memset(wsh[:, :, :, GU + W - 1:WB], 0.0)

            # All big transfers ride one queue in pipeline priority order
            # (in-order arrival, no bandwidth sharing; concurrent queues
            # split the 16 shared DMA engines' bandwidth): shift matrices
            # first (the PE warm-up needs them immediately), band 0 split
            # in two so its normalizer matmuls start ~halfway through the
            # transfer, feature, bands 1-2.
            nc.sync.dma_start(out=smat_t, in_=smat.ap())
            for n0, n1 in ((0, 2), (2, 5), (5, K2)):
                nc.sync.dma_start(out=absa[0][:, n0:n1],
                                  in_=aff_b[0].ap()[:, n0:n1])
            nc.sync.dma_start(out=f0, in_=feat_r)
            for bi in (1, 2):
                nc.sync.dma_start(out=absa[bi], in_=aff_b[bi].ap())

            # PE p-state warm-up: idle matmuls on the (tiny, first-landed)
            # shift matrices keep PE continuously busy from ~8us so the
            # HAM clock is fully ramped (2.4 GHz) when band 0's normalizer
            # matmuls start; cold-start would run them at <=1.2 GHz.
            warm = prepsum.tile([P, 512], dt.float32, tag="pps",
                                name="warm")
            for _ in range(24):
                nc.tensor.matmul(out=warm[:, 0:P], lhsT=smat_t[:, 1, :],
                                 rhs=smat_t[:, 1, :], start=True, stop=True,
                                 skip_group_check=True)

            # ---- per-band normalize + pre-shift, pipelined with DMA ----
            def recip_slot(bi, s):
                lb = TILES[bi][1] + 2
                nc.vector.reciprocal_approx_fast(
                    out=rcp32[bi][:, s, 0:lb], in_=acc[(bi, s)])
                # ACT cast to fp16 so the normalize muls run in 2x mode;
                # slot 0 is duplicated into slot 2 for the direct pairs.
                nc.scalar.copy(out=rcp[bi][:, s, 0:lb],
                               in_=rcp32[bi][:, s, 0:lb])
                if s == 0:
                    nc.scalar.copy(out=rcp[bi][:, 2, 0:lb],
                                   in_=rcp32[bi][:, 0, 0:lb])

            def stage_half(bi, half):
                # One TT per tap-triple: the three maps ride the n axis,
                # the (per-slot) rcp broadcasts across it with stride 0.
                # Full local range incl. overlap columns.
                lb = TILES[bi][1] + 2
                wst, n0, c_src = ((ws03[bi], 0, 0), (ws68[bi], 6, 1))[half]
                rb = rcp[bi][:, c_src, 0:lb].unsqueeze(1)\
                    .broadcast_to([P, 3, lb])
                nc.vector.tensor_tensor(
                    out=wst[:, :, 0:lb],
                    in0=absa[bi][:, n0:n0 + 3, c_src, 0:lb],
                    in1=rb, op=mybir.AluOpType.mult)

            # tap -> (stationary idx, matmul src slot, direct src slot)
            mm_plan = {0: (2, 0, 1), 1: (2, 0, 1), 2: (2, 0, 1),
                       6: (0, 1, 0), 7: (0, 1, 0), 8: (0, 1, 0)}

            def direct_band(bi):
                # Slot-swap halves of the row-shifted taps, then the
                # unshifted middle row (both slots at once).
                t0, L = TILES[bi]
                g0 = t0 - 1
                e0, e1 = t0, t0 + L
                # Taps n and n+6 share cs and their (dst slot, src slot)
                # anti-diagonal pairs become one strided dim on the
                # flattened (tap, slot) axis: wsh (n,0)/(n+6,1) step 13,
                # absa (n,1)/(n+6,0) step 11, rcp slots (s1, s0) = [1:3].
                wflat = wsh.rearrange("p n c j -> p (n c) j")
                aflat = absa[bi].rearrange("p n c j -> p (n c) j")
                lb = L + 2
                for n in (0, 1, 2):
                    cs = n % 3 - 1
                    a0 = max(max(0, cs), e0)
                    a1 = min(W + min(0, cs), e1)
                    l0, l1 = a0 - cs - g0, a1 - cs - g0
                    nc.vector.tensor_mul(
                        wflat[:, 2 * n:2 * n + 14:13, GU + a0:GU + a1],
                        aflat[:, 2 * n + 1:2 * n + 13:11, l0:l1],
                        rcp[bi][:, 1:3, l0:l1])
                for n in (3, 5, 4):
                    cs = n % 3 - 1
                    a0 = max(max(0, cs), e0)
                    a1 = min(W + min(0, cs), e1)
                    nc.vector.tensor_mul(
                        wsh[:, n, :, GU + a0:GU + a1],
                        absa[bi][:, n, :, a0 - cs - g0:a1 - cs - g0],
                        rcp[bi][:, :, a0 - cs - g0:a1 - cs - g0])

            def build_band(bi):
                # Partition-shift halves of the row-shifted taps: PE shift
                # matmul off the staged (normalized) maps + ACT evac.
                t0, L = TILES[bi]
                g0 = t0 - 1
                ws_t = {0: ws03[bi][:, 0], 1: ws03[bi][:, 1],
                        2: ws03[bi][:, 2], 6: ws68[bi][:, 0],
                        7: ws68[bi][:, 1], 8: ws68[bi][:, 2]}
                for n in SHIFT_TAPS:  # S_up x3 then S_dn x3: 2 LDW / band
                    cs = n % 3 - 1
                    sidx = mm_plan[n][0]
                    c_out = 1 if n < 3 else 0
                    jd0, jd1 = max(0, cs), W + min(0, cs)
                    a0, a1 = max(jd0, t0), min(jd1, t0 + L)
                    psb = prepsum.tile([P, 512], dt.float32, tag="pps",
                                       name=f"pps_{n}_{bi}")
                    nc.tensor.matmul(out=psb[:, 0:a1 - a0],
                                     lhsT=smat_t[:, sidx, :],
                                     rhs=ws_t[n][:, a0 - cs - g0:a1 - cs - g0],
                                     start=True, stop=True,
                                     skip_group_check=True)
                    nc.scalar.copy(out=wsh[:, n, c_out, GU + a0:GU + a1],
                                   in_=psb[:, 0:a1 - a0])

            # Emission: normalizer sums band-major (each gated only on its
            # own band's DMA), then per band recip -> stage/direct -> shift
            # matmuls -> iteration 1's TT for that tile.  Band 0 (the
            # critical path to the first TT) finishes slot 0's sum first
            # so its recip/stage chain overlaps slot 1's matmuls.
            def norm_mm(bi, n, s):
                lb = TILES[bi][1] + 2
                nc.tensor.matmul(out=acc[(bi, s)], lhsT=smat_t[:, 1, :],
                                 rhs=absa[bi][:, n, s, 0:lb],
                                 start=(n == 0), stop=(n == K2 - 1),
                                 skip_group_check=True)

            # Iteration 1's tile-0 accumulators REUSE band 0's normalizer
            # PSUM banks (free once its reciprocal has read them), so its
            # matmul phases can interleave with the later bands' builds
            # (otherwise they queue behind ALL band matmuls on cooled-down
            # PE and the loop's second iteration stalls on tile-0's evac).
            pst0 = {c: acc[(0, c)][:, 0:TILES[0][1]] for c in (0, 1)}
            f1 = fpool.tile([P, NS, W], dt.float16, tag="f",
                            name="f1") if times > 1 else None

            for bi in range(3):
                if bi == 0:
                    # Matmuls chase the three DMA chunks; slot 0's sum
                    # finishes first so its recip/stage overlap slot 1's
                    # remaining matmuls.
                    for n in range(5):
                        for s in range(NS):
                            norm_mm(bi, n, s)
                    for n in range(5, K2):
                        norm_mm(bi, n, 0)
                    recip_slot(bi, 0)
                    stage_half(bi, 0)
                    for n in range(5, K2):
                        norm_mm(bi, n, 1)
                    recip_slot(bi, 1)
                    stage_half(bi, 1)
                else:
                    for n in range(K2):
                        for s in range(NS):
                            norm_mm(bi, n, s)
                    for s in range(NS):
                        recip_slot(bi, s)
                    for half in (0, 1):
                        stage_half(bi, half)
                direct_band(bi)
                build_band(bi)
                emit_tt(z0, f0, *TILES[bi])
                # Iteration 1's tile-0 matmul phases interleave here, but
                # only AFTER the next band's normalizer matmuls have been
                # emitted: PE executes in order, and these gate on the
                # (much later) TT0/TT1, so emitting them earlier would
                # block the bands' independent PE work behind the stall.
                if bi == 1 and times > 1:
                    # Open tile 0's accumulation (dj<=1 taps read only
                    # tile-0 z columns, available since TT0).
                    for c in (0, 1):
                        emit_mms(z0, pst0[c], c, *TILES[0], dj01_t0[c],
                                 start=True, stop=False)
                elif bi == 2 and times > 1:
                    # Close tile 0 (dj==2 taps read one tile-1 column,
                    # available since TT1) and evacuate it as iteration
                    # 2's first feature tile.
                    p0, pl = TILES[0]
                    for c in (0, 1):
                        emit_mms(z0, pst0[c], c, p0, pl, dj2[c],
                                 start=False, stop=True)
                        nc.scalar.copy(out=f1[:, c, p0:p0 + pl],
                                       in_=pst0[c])

        psump = ctx.enter_context(tc.tile_pool(name="psum", bufs=8,
                                               space="PSUM"))
        loopp = ctx.enter_context(tc.tile_pool(name="loop", bufs=1))
        z1 = loopp.tile([P, K2, NS, WB], dt.float16, name="z1")
        zbufs = [z0, z1]
        # Final-iteration staging: evacuated per PSUM bank, DMA'd per bank.
        fout = loopp.tile([P, NS, W], dt.float16)

        # ---- stencil iterations ----
        # Matmuls for PSUM bank (c, tile t) are split by column dependency:
        # taps with dj<=1 only read z columns from tiles <= t, so they can
        # start as soon as the TT covering tile t lands; dj==2 taps read
        # one column of tile t+1.  This keeps PE busy throughout the DVE
        # phase (HAM stays at full clock).
        f_cur = f0
        for k in range(times):
            last = k == times - 1
            z = zbufs[k % 2]
            # k == 0's TTs (and its tile-0 matmul phases) were emitted in
            # the preamble band pipeline; its f_next was preallocated.
            pre_t0 = k == 0 and times > 1
            # The final iteration uses a finer last tile (LAST_TILES) so
            # its trailing matmul+evac+store chain is shorter.  (k == 0 is
            # never 'last' here with times > 1; for times == 1 the plain
            # TILES path is kept.)
            tiles_k = LAST_TILES if (last and k > 0) else TILES
            if k > 0:
                for (t0, L) in tiles_k:
                    emit_tt(z, f_cur, t0, L)
            if last:
                f_next = None
            elif pre_t0:
                f_next = f1
            else:
                f_next = fpool.tile([P, NS, W], dt.float16, tag="f")

            pst = {}
            for ti, (t0, L) in enumerate(tiles_k):
                if pre_t0 and ti == 0:
                    continue
                for c in (0, 1):
                    pst[(c, ti)] = psump.tile([P, L], dt.float32, tag="ps",
                                              name=f"ps_{c}_{ti}")

            def finish_bank(c, ti):
                # Last iteration: ACT evacuates the finished bank as fp16
                # and its DMA to DRAM fires immediately (per bank, so the
                # store pipelines with the remaining banks' matmuls);
                # otherwise ACT evacuates it as the next iteration's fp16
                # feature tile.
                p0, pl = tiles_k[ti]
                if last:
                    nc.scalar.copy(out=fout[:, c, p0:p0 + pl],
                                   in_=pst[(c, ti)])
                    # Hardware-DGE queues (sync/scalar): gpsimd's software
                    # DGE costs a ~2.5us drain in the teardown.
                    eng = nc.sync if c == 0 else nc.scalar
                    eng.dma_start(out=out_r[:, c, p0:p0 + pl],
                                  in_=fout[:, c, p0:p0 + pl])
                else:
                    nc.scalar.copy(out=f_next[:, c, p0:p0 + pl],
                                   in_=pst[(c, ti)])

            # Phase ti: finish banks of tile ti-1 (dj2 taps + evacuate),
            # then open banks of tile ti (dj01 taps).
            nt = len(tiles_k)
            for ti, (t0, L) in enumerate(tiles_k):
                if ti > 0 and not (pre_t0 and ti == 1):
                    p0, pl = tiles_k[ti - 1]
                    for c in (0, 1):
                        emit_mms(z, pst[(c, ti - 1)], c, p0, pl, dj2[c],
                                 start=False, stop=True)
                        finish_bank(c, ti - 1)
                if not (pre_t0 and ti == 0):
                    for c in (0, 1):
                        emit_mms(z, pst[(c, ti)], c, t0, L,
                                 dj01_t0[c] if ti == 0 else dj01[c],
                                 start=True, stop=False)
            t0, L = tiles_k[-1]
            for c in (0, 1):
                emit_mms(z, pst[(c, nt - 1)], c, t0, L, dj2[c], start=False,
                         stop=True)
                finish_bank(c, nt - 1)
            f_cur = f_next

    nc._dbg = {'wsh': wsh, 'smat_t': smat_t}
    nc.finalize()
    return nc


def _get_program(times: int):
    if times not in _CACHE:
        _CACHE[times] = build_program(times)
    return _CACHE[times]


def _in_maps(affinity: np.ndarray, feature: np.ndarray):
    sm = _shift_mats()
    ab = np.abs(affinity).astype(np.float16)
    maps = []
    for b in range(B):
        # [9, 256, 1216] -> [128, 9, 2, 1216] fp16 (the SBUF layout)
        a4 = np.ascontiguousarray(
            ab[b].reshape(K2, P, NS, W).transpose(1, 0, 2, 3))
        m = {
            "feat": np.ascontiguousarray(feature[b].reshape(H, W),
                                         dtype=np.float16),
            "smat": sm,
        }
        for bi, (t0, L) in enumerate(TILES):
            cols = np.clip(np.arange(t0 - 1, t0 + L + 1), 0, W - 1)
            m[f"aff{bi}"] = np.ascontiguousarray(a4[:, :, :, cols])
        maps.append(m)
    return maps


def _run(affinity, feature, times, trace=False):
    from concourse.bass_utils import run_bass_kernel_spmd

    nc = _get_program(int(times))
    res = run_bass_kernel_spmd(nc, _in_maps(affinity, feature),
                               core_ids=list(range(B)), trace=trace)
    outs = np.stack([np.asarray(res.results[b]["out"]) for b in range(B)])
    return outs.reshape(B, 1, H, W).astype(np.float32), res


def kernel(affinity, feature, times) -> np.ndarray:
    affinity = np.asarray(affinity)
    feature = np.asarray(feature)
    assert affinity.shape == (B, K2, H, W), affinity.shape
    assert feature.shape[0] == B and feature.shape[-2:] == (H, W)
    out, _ = _run(affinity, feature, int(times))
    return out
